# revision 30
# baseline (speedup 1.0000x reference)
"""Trainium2 Bass kernel for conv-qkv rank-1 attention (bf16 pipeline).

out = gamma * q * sum(k*v) + x, where q,k,v are per-time-slice 3x3 convs
(C=64 -> C=64) of x [B=8, C=64, T=16, W=64, H=64].

Sharding: data-parallel over B across 8 cores (1 example/core), conv
weights replicated. No cross-core communication.

Per-core schedule: T slices in pairs; slice t on SBUF partitions 0-63,
slice t+1 on 64-127 -> two concurrent PE row-group chains (K=64), which
maxes the array fill rate (1 col/cycle/chain). All matmuls are uniform
64x128 stationaries (geometry changes stall the array ~300ns).
Everything streams bf16, PSUM accumulates f32:
  - x is staged twice (interior at even and odd column offsets) so every
    3x3 tap window is 4B-aligned -- unaligned bf16 moving operands cost
    ~20% fill rate.
  - chain-lo stationary [Wq|Wk] (q_t -> psum parts 0-63, aligned with
    x_t), chain-hi [Wk|Wq]; v uses zero-padded [0|Wv] / [Wv|0] into two
    psum tiles (4 tiles = 8 banks, double buffered).
  - Biases fold into the PSUM->SBUF evictions (Identity activation with
    per-partition bias), which also downcast to bf16. No bias tap.
  - kv mult+pixel-sum: one DVE STT per block half with hw accumulator.
  - out = q*(gamma*s) + x: 2-block-chunk DVE STTs, bf16 in/out; host
    upcasts. The writeback of pair p overlaps pair p+1's matmuls on the
    DVE queue. Host-padded x keeps loads as single contiguous DMAs.
"""

import numpy as np
import ml_dtypes

import concourse.bacc as bacc
import concourse.bass as bass
import concourse.mybir as mybir
import concourse.tile as tile
from concourse import bass_utils

F32 = mybir.dt.float32
BF16 = mybir.dt.bfloat16
ALU = mybir.AluOpType
ACT = mybir.ActivationFunctionType
NPBF16 = np.dtype(ml_dtypes.bfloat16)

B, C, T, W, H = 8, 64, 16, 64, 64
WP, HP = W + 2, H + 4            # pad rows [1,65); cols [2,66) / [3,67)
NPAIR = T // 2
RB = 8                           # W-rows per pixel block
NBLK = W // RB
BN = RB * H                      # moving free dim per matmul (512)
NTAP = 9
QC = 2                           # blocks per out-writeback chunk


def _pack_weights(wq, wk, wv):
    def taps(w):  # [O, I, 1, 3, 3] -> [I, 9, O]
        return np.ascontiguousarray(
            np.asarray(w, np.float32).reshape(C, C, 9).transpose(1, 2, 0))

    wq_t, wk_t, wv_t = taps(wq), taps(wk), taps(wv)
    wqk = np.zeros((128, NTAP, 128), np.float32)
    wqk[0:64, :, 0:64] = wq_t
    wqk[0:64, :, 64:128] = wk_t
    wqk[64:128, :, 0:64] = wk_t
    wqk[64:128, :, 64:128] = wq_t
    # v: chain-lo -> psum parts 64-127 (with k_t), chain-hi -> parts 0-63
    wv2 = np.zeros((128, NTAP, 128), np.float32)
    wv2[0:64, :, 64:128] = wv_t
    wv2[64:128, :, 0:64] = wv_t
    return wqk.astype(NPBF16), wv2.astype(NPBF16)


def _emit(nc, tc, xe_d, xo_d, wqk_d, wv_d, gam_d, blo_d, bhi_d, bvv_d,
          out_d, ctx):
    const = ctx.enter_context(tc.tile_pool(name="const", bufs=1))
    state = ctx.enter_context(tc.tile_pool(name="state", bufs=1))
    psum = ctx.enter_context(
        tc.tile_pool(name="psum", bufs=2, space=bass.MemorySpace.PSUM))
    vpool = ctx.enter_context(tc.tile_pool(name="vpool", bufs=3))

    wqk_t = const.tile([128, NTAP, 128], BF16, tag="wqk")
    wv_t = const.tile([128, NTAP, 128], BF16, tag="wv")
    gam_t = const.tile([128, 1], F32, tag="gam")
    blo_t = const.tile([128, 1], F32, tag="blo")
    bhi_t = const.tile([128, 1], F32, tag="bhi")
    bvv_t = const.tile([128, 1], F32, tag="bvv")

    # tap-0 stationary first: it gates the very first matmul
    nc.sync.dma_start(wqk_t[:, 0, :], wqk_d[:, 0, :])

    xe = [state.tile([128, WP, HP], BF16, tag=f"xe{i}", name=f"xe{i}")
          for i in range(3)]
    xo = [state.tile([128, WP, HP], BF16, tag=f"xo{i}", name=f"xo{i}")
          for i in range(3)]
    qk_lo = [state.tile([128, NBLK, BN], BF16, tag=f"qlo{i}", name=f"qlo{i}")
             for i in range(2)]
    qk_hi = [state.tile([128, NBLK, BN], BF16, tag=f"qhi{i}", name=f"qhi{i}")
             for i in range(2)]
    ot = [state.tile([128, NBLK, BN], BF16, tag=f"ot{i}", name=f"ot{i}")
          for i in range(2)]
    scr = state.tile([128, BN], BF16, tag="scr")
    sacc = [state.tile([128, NBLK], F32, tag=f"sa{i}", name=f"sa{i}")
            for i in range(2)]
    sful = [state.tile([128, 1], F32, tag=f"sf{i}", name=f"sf{i}")
            for i in range(2)]
    gsw = [state.tile([128, 1], F32, tag=f"gw{i}", name=f"gw{i}")
           for i in range(2)]

    def load_pair(p):
        te, to = xe[p % 3], xo[p % 3]
        nc.sync.dma_start(te[0:64], xe_d[2 * p])
        nc.sync.dma_start(te[64:128], xe_d[2 * p + 1])
        nc.sync.dma_start(to[0:64], xo_d[2 * p])
        nc.sync.dma_start(to[64:128], xo_d[2 * p + 1])

    # pair 0 loads in row-chunks in need-order: block j consumes padded
    # rows j*8..j*8+9, so the stream can start after ~0.6MB instead of
    # waiting for all 2.9MB of startup transfers (~200GB/s effective)
    p0_bounds = [(0, 10), (10, 26), (26, 42), (42, 58), (58, 66)]
    for ci, (r0, r1) in enumerate(p0_bounds):
        nc.sync.dma_start(xo[0][0:64, r0:r1, :], xo_d[0, :, r0:r1, :])
        nc.sync.dma_start(xo[0][64:128, r0:r1, :], xo_d[1, :, r0:r1, :])
        nc.sync.dma_start(xe[0][0:64, r0:r1, :], xe_d[0, :, r0:r1, :])
        nc.sync.dma_start(xe[0][64:128, r0:r1, :], xe_d[1, :, r0:r1, :])
        if ci == 0:
            nc.sync.dma_start(wqk_t[:, 1:NTAP, :], wqk_d[:, 1:NTAP, :])
            nc.sync.dma_start(wv_t[:], wv_d[:])    # needed at slot 9
    nc.sync.dma_start(gam_t[:], gam_d[:])
    nc.sync.dma_start(blo_t[:], blo_d[:])
    nc.sync.dma_start(bhi_t[:], bhi_d[:])
    nc.sync.dma_start(bvv_t[:], bvv_d[:])
    if NPAIR > 1:
        load_pair(1)

    for p in range(NPAIR):
        pb = p % 2
        xe_, xo_ = xe[p % 3], xo[p % 3]
        qlo_, qhi_, ot_ = qk_lo[pb], qk_hi[pb], ot[pb]

        if p + 2 < NPAIR:
            load_pair(p + 2)

        for j in range(NBLK):
            pqk_lo = psum.tile([128, BN], F32, tag="pqk_lo")
            pqk_hi = psum.tile([128, BN], F32, tag="pqk_hi")
            pvv_lo = psum.tile([128, BN], F32, tag="pvv_lo", name="pvv_lo")
            pvv_hi = psum.tile([128, BN], F32, tag="pvv_hi", name="pvv_hi")

            def rhs(half, tap):
                dy, dx = tap // 3, tap % 3
                r0 = j * RB + dy
                base = 64 * half
                if dx == 1:
                    return xe_[base:base + 64, r0:r0 + RB, 2:2 + H]
                if dx == 0:
                    return xo_[base:base + 64, r0:r0 + RB, 2:2 + H]
                return xo_[base:base + 64, r0:r0 + RB, 4:4 + H]

            for tap in range(NTAP):
                st, sp = tap == 0, tap == NTAP - 1
                nc.tensor.matmul(pqk_lo[:, :], wqk_t[0:64, tap, :],
                                 rhs(0, tap), start=st, stop=sp)
                nc.tensor.matmul(pqk_hi[:, :], wqk_t[64:128, tap, :],
                                 rhs(1, tap), start=st, stop=sp)
            for tap in range(NTAP):
                st, sp = tap == 0, tap == NTAP - 1
                nc.tensor.matmul(pvv_lo[:, :], wv_t[0:64, tap, :],
                                 rhs(0, tap), start=st, stop=sp)
                nc.tensor.matmul(pvv_hi[:, :], wv_t[64:128, tap, :],
                                 rhs(1, tap), start=st, stop=sp)

            # evict psum -> bf16 sbuf, adding conv biases (per-partition)
            nc.scalar.activation(qlo_[:, j, :], pqk_lo[:, :], ACT.Identity,
                                 bias=blo_t[:, 0:1])
            nc.scalar.activation(qhi_[:, j, :], pqk_hi[:, :], ACT.Identity,
                                 bias=bhi_t[:, 0:1])
            vsb = vpool.tile([128, BN], BF16, tag="vsb", name="vsb")
            nc.scalar.activation(vsb[64:128, :], pvv_lo[64:128, :],
                                 ACT.Identity, bias=bvv_t[64:128, 0:1])
            nc.scalar.activation(vsb[0:64, :], pvv_hi[0:64, :],
                                 ACT.Identity, bias=bvv_t[0:64, 0:1])

            # fused k*v multiply + pixel-sum
            nc.vector.scalar_tensor_tensor(
                out=scr[64:128, :], in0=qlo_[64:128, j, :], scalar=1.0,
                in1=vsb[64:128, :], op0=ALU.mult, op1=ALU.mult,
                accum_out=sacc[pb][64:128, j:j + 1])
            nc.vector.scalar_tensor_tensor(
                out=scr[0:64, :], in0=qhi_[0:64, j, :], scalar=1.0,
                in1=vsb[0:64, :], op0=ALU.mult, op1=ALU.mult,
                accum_out=sacc[pb][0:64, j:j + 1])

        nc.vector.reduce_sum(sful[pb][:, :], sacc[pb][:, :],
                             axis=mybir.AxisListType.X)
        nc.vector.tensor_scalar_mul(sful[pb][:, :], sful[pb][:, :],
                                    gam_t[:, 0:1])
        # gs accumulated on k's partitions = complement of q's: swap halves
        nc.scalar.copy(gsw[pb][0:64, :], sful[pb][64:128, :])
        nc.scalar.copy(gsw[pb][64:128, :], sful[pb][0:64, :])

        for m in range(0, NBLK, QC):
            # out = q * (gamma*s) + x, fused (bf16), QC blocks per op
            r0 = 1 + m * RB
            nc.vector.scalar_tensor_tensor(
                out=ot_[0:64, m:m + QC, :],
                in0=qlo_[0:64, m:m + QC, :],
                scalar=gsw[pb][0:64, 0:1],
                in1=xe_[0:64, r0:r0 + QC * RB, 2:2 + H],
                op0=ALU.mult, op1=ALU.add)
            nc.vector.scalar_tensor_tensor(
                out=ot_[64:128, m:m + QC, :],
                in0=qhi_[64:128, m:m + QC, :],
                scalar=gsw[pb][64:128, 0:1],
                in1=xe_[64:128, r0:r0 + QC * RB, 2:2 + H],
                op0=ALU.mult, op1=ALU.add)
            nc.sync.dma_start(out_d[2 * p, :, m * BN:(m + QC) * BN],
                              ot_[0:64, m:m + QC, :])
            nc.sync.dma_start(
                out_d[2 * p + 1, :, m * BN:(m + QC) * BN],
                ot_[64:128, m:m + QC, :])


_CACHE = {}


def _build():
    if "nc" in _CACHE:
        return _CACHE["nc"]
    nc = bacc.Bacc("TRN2", target_bir_lowering=False, debug=False,
                   enable_asserts=False, num_devices=8)
    xe_d = nc.dram_tensor("xe16", (T, C, WP, HP), BF16,
                          kind="ExternalInput").ap()
    xo_d = nc.dram_tensor("xo16", (T, C, WP, HP), BF16,
                          kind="ExternalInput").ap()
    wqk_d = nc.dram_tensor("wqk", (128, NTAP, 128), BF16,
                           kind="ExternalInput").ap()
    wv_d = nc.dram_tensor("wv2", (128, NTAP, 128), BF16,
                          kind="ExternalInput").ap()
    gam_d = nc.dram_tensor("gamma_bc", (128, 1), F32,
                           kind="ExternalInput").ap()
    blo_d = nc.dram_tensor("b_lo", (128, 1), F32, kind="ExternalInput").ap()
    bhi_d = nc.dram_tensor("b_hi", (128, 1), F32, kind="ExternalInput").ap()
    bvv_d = nc.dram_tensor("b_vv", (128, 1), F32, kind="ExternalInput").ap()
    out_d = nc.dram_tensor("out", (T, C, W * H), BF16,
                           kind="ExternalOutput").ap()
    from contextlib import ExitStack
    with tile.TileContext(nc) as tc, ExitStack() as ctx:
        _emit(nc, tc, xe_d, xo_d, wqk_d, wv_d, gam_d, blo_d, bhi_d, bvv_d,
              out_d, ctx)
    nc.compile()
    _CACHE["nc"] = nc
    return nc


def run_spmd(x, wq, wk, wv, bq, bk, bv, gamma, trace=False, **kw):
    nc = _build()
    wqk, wv2 = _pack_weights(wq, wk, wv)
    bq = np.asarray(bq, np.float32).reshape(C)
    bk = np.asarray(bk, np.float32).reshape(C)
    bv = np.asarray(bv, np.float32).reshape(C)
    blo = np.concatenate([bq, bk]).reshape(128, 1)
    bhi = np.concatenate([bk, bq]).reshape(128, 1)
    bvv = np.concatenate([bv, bv]).reshape(128, 1)
    gam = np.full((128, 1), np.float32(np.asarray(gamma).reshape(-1)[0]),
                  np.float32)
    x = np.asarray(x, np.float32)
    in_maps = []
    for b in range(B):
        xt = x[b].transpose(1, 0, 2, 3).astype(NPBF16)
        xe = np.zeros((T, C, WP, HP), NPBF16)
        xe[:, :, 1:1 + W, 2:2 + H] = xt
        xo = np.zeros((T, C, WP, HP), NPBF16)
        xo[:, :, 1:1 + W, 3:3 + H] = xt
        in_maps.append({"xe16": xe, "xo16": xo, "wqk": wqk, "wv2": wv2,
                        "gamma_bc": gam, "b_lo": blo, "b_hi": bhi,
                        "b_vv": bvv})
    res = bass_utils.run_bass_kernel_spmd(
        nc, in_maps, core_ids=list(range(B)), trace=trace, **kw)
    out = np.stack(
        [res.results[b]["out"].astype(np.float32)
         .reshape(T, C, W, H).transpose(1, 0, 2, 3) for b in range(B)],
        axis=0)
    return out, res


def kernel(x, wq, wk, wv, bq, bk, bv, gamma):
    out, _ = run_spmd(x, wq, wk, wv, bq, bk, bv, gamma)
    return out


# revision 32
# speedup vs baseline: 1.0002x; 1.0002x over previous
"""Trainium2 Bass kernel for conv-qkv rank-1 attention (bf16 pipeline).

out = gamma * q * sum(k*v) + x, where q,k,v are per-time-slice 3x3 convs
(C=64 -> C=64) of x [B=8, C=64, T=16, W=64, H=64].

Sharding: data-parallel over B across 8 cores (1 example/core), conv
weights replicated. No cross-core communication.

Per-core schedule: T slices in pairs; slice t on SBUF partitions 0-63,
slice t+1 on 64-127 -> two concurrent PE row-group chains (K=64), which
maxes the array fill rate (1 col/cycle/chain). All matmuls are uniform
64x128 stationaries (geometry changes stall the array ~300ns).
Everything streams bf16, PSUM accumulates f32:
  - x is staged twice (interior at even and odd column offsets) so every
    3x3 tap window is 4B-aligned -- unaligned bf16 moving operands cost
    ~20% fill rate.
  - chain-lo stationary [Wq|Wk] (q_t -> psum parts 0-63, aligned with
    x_t), chain-hi [Wk|Wq]; v uses zero-padded [0|Wv] / [Wv|0] into two
    psum tiles (4 tiles = 8 banks, double buffered).
  - Biases fold into the PSUM->SBUF evictions (Identity activation with
    per-partition bias), which also downcast to bf16. No bias tap.
  - kv mult+pixel-sum: one DVE STT per block half with hw accumulator.
  - out = q*(gamma*s) + x: 2-block-chunk DVE STTs, bf16 in/out; host
    upcasts. The writeback of pair p overlaps pair p+1's matmuls on the
    DVE queue. Host-padded x keeps loads as single contiguous DMAs.
"""

import numpy as np
import ml_dtypes

import concourse.bacc as bacc
import concourse.bass as bass
import concourse.mybir as mybir
import concourse.tile as tile
from concourse import bass_utils

F32 = mybir.dt.float32
BF16 = mybir.dt.bfloat16
ALU = mybir.AluOpType
ACT = mybir.ActivationFunctionType
NPBF16 = np.dtype(ml_dtypes.bfloat16)

B, C, T, W, H = 8, 64, 16, 64, 64
WP, HP = W + 2, H + 4            # pad rows [1,65); cols [2,66) / [3,67)
NPAIR = T // 2
RB = 8                           # W-rows per pixel block
NBLK = W // RB
BN = RB * H                      # moving free dim per matmul (512)
NTAP = 9
QC = 2                           # blocks per out-writeback chunk


def _pack_weights(wq, wk, wv):
    def taps(w):  # [O, I, 1, 3, 3] -> [I, 9, O]
        return np.ascontiguousarray(
            np.asarray(w, np.float32).reshape(C, C, 9).transpose(1, 2, 0))

    wq_t, wk_t, wv_t = taps(wq), taps(wk), taps(wv)
    wqk = np.zeros((128, NTAP, 128), np.float32)
    wqk[0:64, :, 0:64] = wq_t
    wqk[0:64, :, 64:128] = wk_t
    wqk[64:128, :, 0:64] = wk_t
    wqk[64:128, :, 64:128] = wq_t
    # v: chain-lo -> psum parts 64-127 (with k_t), chain-hi -> parts 0-63
    wv2 = np.zeros((128, NTAP, 128), np.float32)
    wv2[0:64, :, 64:128] = wv_t
    wv2[64:128, :, 0:64] = wv_t
    return wqk.astype(NPBF16), wv2.astype(NPBF16)


def _emit(nc, tc, xe_d, xo_d, wqk_d, wv_d, gam_d, blo_d, bhi_d, bvv_d,
          out_d, ctx):
    const = ctx.enter_context(tc.tile_pool(name="const", bufs=1))
    state = ctx.enter_context(tc.tile_pool(name="state", bufs=1))
    psum = ctx.enter_context(
        tc.tile_pool(name="psum", bufs=2, space=bass.MemorySpace.PSUM))
    vpool = ctx.enter_context(tc.tile_pool(name="vpool", bufs=3))

    wqk_t = const.tile([128, NTAP, 128], BF16, tag="wqk")
    wv_t = const.tile([128, NTAP, 128], BF16, tag="wv")
    gam_t = const.tile([128, 1], F32, tag="gam")
    blo_t = const.tile([128, 1], F32, tag="blo")
    bhi_t = const.tile([128, 1], F32, tag="bhi")
    bvv_t = const.tile([128, 1], F32, tag="bvv")

    nc.sync.dma_start(wqk_t[:], wqk_d[:])

    xe = [state.tile([128, WP, HP], BF16, tag=f"xe{i}", name=f"xe{i}")
          for i in range(3)]
    xo = [state.tile([128, WP, HP], BF16, tag=f"xo{i}", name=f"xo{i}")
          for i in range(3)]
    qk_lo = [state.tile([128, NBLK, BN], BF16, tag=f"qlo{i}", name=f"qlo{i}")
             for i in range(2)]
    qk_hi = [state.tile([128, NBLK, BN], BF16, tag=f"qhi{i}", name=f"qhi{i}")
             for i in range(2)]
    ot = [state.tile([128, NBLK, BN], BF16, tag=f"ot{i}", name=f"ot{i}")
          for i in range(2)]
    scr = state.tile([128, BN], BF16, tag="scr")
    sacc = [state.tile([128, NBLK], F32, tag=f"sa{i}", name=f"sa{i}")
            for i in range(2)]
    sful = [state.tile([128, 1], F32, tag=f"sf{i}", name=f"sf{i}")
            for i in range(2)]
    gsw = [state.tile([128, 1], F32, tag=f"gw{i}", name=f"gw{i}")
           for i in range(2)]

    def load_pair(p):
        te, to = xe[p % 3], xo[p % 3]
        nc.sync.dma_start(te[0:64], xe_d[2 * p])
        nc.sync.dma_start(te[64:128], xe_d[2 * p + 1])
        nc.sync.dma_start(to[0:64], xo_d[2 * p])
        nc.sync.dma_start(to[64:128], xo_d[2 * p + 1])

    # pair 0 loads in row-chunks in need-order: block j consumes padded
    # rows j*8..j*8+9, so the stream can start after ~0.6MB instead of
    # waiting for all 2.9MB of startup transfers (~200GB/s effective)
    p0_bounds = [(0, 18), (18, 34), (34, 50), (50, 66)]
    for ci, (r0, r1) in enumerate(p0_bounds):
        nc.sync.dma_start(xo[0][0:64, r0:r1, :], xo_d[0, :, r0:r1, :])
        nc.sync.dma_start(xo[0][64:128, r0:r1, :], xo_d[1, :, r0:r1, :])
        nc.sync.dma_start(xe[0][0:64, r0:r1, :], xe_d[0, :, r0:r1, :])
        nc.sync.dma_start(xe[0][64:128, r0:r1, :], xe_d[1, :, r0:r1, :])
        if ci == 0:
            nc.sync.dma_start(wv_t[:], wv_d[:])    # needed at slot 9
    nc.sync.dma_start(gam_t[:], gam_d[:])
    nc.sync.dma_start(blo_t[:], blo_d[:])
    nc.sync.dma_start(bhi_t[:], bhi_d[:])
    nc.sync.dma_start(bvv_t[:], bvv_d[:])
    if NPAIR > 1:
        load_pair(1)

    for p in range(NPAIR):
        pb = p % 2
        xe_, xo_ = xe[p % 3], xo[p % 3]
        qlo_, qhi_, ot_ = qk_lo[pb], qk_hi[pb], ot[pb]

        if p + 2 < NPAIR:
            load_pair(p + 2)

        for j in range(NBLK):
            pqk_lo = psum.tile([128, BN], F32, tag="pqk_lo")
            pqk_hi = psum.tile([128, BN], F32, tag="pqk_hi")
            pvv_lo = psum.tile([128, BN], F32, tag="pvv_lo", name="pvv_lo")
            pvv_hi = psum.tile([128, BN], F32, tag="pvv_hi", name="pvv_hi")

            def rhs(half, tap):
                dy, dx = tap // 3, tap % 3
                r0 = j * RB + dy
                base = 64 * half
                if dx == 1:
                    return xe_[base:base + 64, r0:r0 + RB, 2:2 + H]
                if dx == 0:
                    return xo_[base:base + 64, r0:r0 + RB, 2:2 + H]
                return xo_[base:base + 64, r0:r0 + RB, 4:4 + H]

            for tap in range(NTAP):
                st, sp = tap == 0, tap == NTAP - 1
                nc.tensor.matmul(pqk_lo[:, :], wqk_t[0:64, tap, :],
                                 rhs(0, tap), start=st, stop=sp)
                nc.tensor.matmul(pqk_hi[:, :], wqk_t[64:128, tap, :],
                                 rhs(1, tap), start=st, stop=sp)
            for tap in range(NTAP):
                st, sp = tap == 0, tap == NTAP - 1
                nc.tensor.matmul(pvv_lo[:, :], wv_t[0:64, tap, :],
                                 rhs(0, tap), start=st, stop=sp)
                nc.tensor.matmul(pvv_hi[:, :], wv_t[64:128, tap, :],
                                 rhs(1, tap), start=st, stop=sp)

            # evict psum -> bf16 sbuf, adding conv biases (per-partition)
            nc.scalar.activation(qlo_[:, j, :], pqk_lo[:, :], ACT.Identity,
                                 bias=blo_t[:, 0:1])
            nc.scalar.activation(qhi_[:, j, :], pqk_hi[:, :], ACT.Identity,
                                 bias=bhi_t[:, 0:1])
            vsb = vpool.tile([128, BN], BF16, tag="vsb", name="vsb")
            nc.scalar.activation(vsb[64:128, :], pvv_lo[64:128, :],
                                 ACT.Identity, bias=bvv_t[64:128, 0:1])
            nc.scalar.activation(vsb[0:64, :], pvv_hi[0:64, :],
                                 ACT.Identity, bias=bvv_t[0:64, 0:1])

            # fused k*v multiply + pixel-sum
            nc.vector.scalar_tensor_tensor(
                out=scr[64:128, :], in0=qlo_[64:128, j, :], scalar=1.0,
                in1=vsb[64:128, :], op0=ALU.mult, op1=ALU.mult,
                accum_out=sacc[pb][64:128, j:j + 1])
            nc.vector.scalar_tensor_tensor(
                out=scr[0:64, :], in0=qhi_[0:64, j, :], scalar=1.0,
                in1=vsb[0:64, :], op0=ALU.mult, op1=ALU.mult,
                accum_out=sacc[pb][0:64, j:j + 1])

        nc.vector.reduce_sum(sful[pb][:, :], sacc[pb][:, :],
                             axis=mybir.AxisListType.X)
        nc.vector.tensor_scalar_mul(sful[pb][:, :], sful[pb][:, :],
                                    gam_t[:, 0:1])
        # gs accumulated on k's partitions = complement of q's: swap halves
        nc.scalar.copy(gsw[pb][0:64, :], sful[pb][64:128, :])
        nc.scalar.copy(gsw[pb][64:128, :], sful[pb][0:64, :])

        # last pair: one full-slice op per half -- its writeback is the
        # exposed tail, and fewer serial DVE ops end sooner
        qc = NBLK if p == NPAIR - 1 else QC
        for m in range(0, NBLK, qc):
            # out = q * (gamma*s) + x, fused (bf16), qc blocks per op
            r0 = 1 + m * RB
            nc.vector.scalar_tensor_tensor(
                out=ot_[0:64, m:m + qc, :],
                in0=qlo_[0:64, m:m + qc, :],
                scalar=gsw[pb][0:64, 0:1],
                in1=xe_[0:64, r0:r0 + qc * RB, 2:2 + H],
                op0=ALU.mult, op1=ALU.add)
            nc.vector.scalar_tensor_tensor(
                out=ot_[64:128, m:m + qc, :],
                in0=qhi_[64:128, m:m + qc, :],
                scalar=gsw[pb][64:128, 0:1],
                in1=xe_[64:128, r0:r0 + qc * RB, 2:2 + H],
                op0=ALU.mult, op1=ALU.add)
            nc.sync.dma_start(out_d[2 * p, :, m * BN:(m + qc) * BN],
                              ot_[0:64, m:m + qc, :])
            nc.sync.dma_start(
                out_d[2 * p + 1, :, m * BN:(m + qc) * BN],
                ot_[64:128, m:m + qc, :])


_CACHE = {}


def _build():
    if "nc" in _CACHE:
        return _CACHE["nc"]
    nc = bacc.Bacc("TRN2", target_bir_lowering=False, debug=False,
                   enable_asserts=False, num_devices=8)
    xe_d = nc.dram_tensor("xe16", (T, C, WP, HP), BF16,
                          kind="ExternalInput").ap()
    xo_d = nc.dram_tensor("xo16", (T, C, WP, HP), BF16,
                          kind="ExternalInput").ap()
    wqk_d = nc.dram_tensor("wqk", (128, NTAP, 128), BF16,
                           kind="ExternalInput").ap()
    wv_d = nc.dram_tensor("wv2", (128, NTAP, 128), BF16,
                          kind="ExternalInput").ap()
    gam_d = nc.dram_tensor("gamma_bc", (128, 1), F32,
                           kind="ExternalInput").ap()
    blo_d = nc.dram_tensor("b_lo", (128, 1), F32, kind="ExternalInput").ap()
    bhi_d = nc.dram_tensor("b_hi", (128, 1), F32, kind="ExternalInput").ap()
    bvv_d = nc.dram_tensor("b_vv", (128, 1), F32, kind="ExternalInput").ap()
    out_d = nc.dram_tensor("out", (T, C, W * H), BF16,
                           kind="ExternalOutput").ap()
    from contextlib import ExitStack
    with tile.TileContext(nc) as tc, ExitStack() as ctx:
        _emit(nc, tc, xe_d, xo_d, wqk_d, wv_d, gam_d, blo_d, bhi_d, bvv_d,
              out_d, ctx)
    nc.compile()
    _CACHE["nc"] = nc
    return nc


def run_spmd(x, wq, wk, wv, bq, bk, bv, gamma, trace=False, **kw):
    nc = _build()
    wqk, wv2 = _pack_weights(wq, wk, wv)
    bq = np.asarray(bq, np.float32).reshape(C)
    bk = np.asarray(bk, np.float32).reshape(C)
    bv = np.asarray(bv, np.float32).reshape(C)
    blo = np.concatenate([bq, bk]).reshape(128, 1)
    bhi = np.concatenate([bk, bq]).reshape(128, 1)
    bvv = np.concatenate([bv, bv]).reshape(128, 1)
    gam = np.full((128, 1), np.float32(np.asarray(gamma).reshape(-1)[0]),
                  np.float32)
    x = np.asarray(x, np.float32)
    in_maps = []
    for b in range(B):
        xt = x[b].transpose(1, 0, 2, 3).astype(NPBF16)
        xe = np.zeros((T, C, WP, HP), NPBF16)
        xe[:, :, 1:1 + W, 2:2 + H] = xt
        xo = np.zeros((T, C, WP, HP), NPBF16)
        xo[:, :, 1:1 + W, 3:3 + H] = xt
        in_maps.append({"xe16": xe, "xo16": xo, "wqk": wqk, "wv2": wv2,
                        "gamma_bc": gam, "b_lo": blo, "b_hi": bhi,
                        "b_vv": bvv})
    res = bass_utils.run_bass_kernel_spmd(
        nc, in_maps, core_ids=list(range(B)), trace=trace, **kw)
    out = np.stack(
        [res.results[b]["out"].astype(np.float32)
         .reshape(T, C, W, H).transpose(1, 0, 2, 3) for b in range(B)],
        axis=0)
    return out, res


def kernel(x, wq, wk, wv, bq, bk, bv, gamma):
    out, _ = run_spmd(x, wq, wk, wv, bq, bk, bv, gamma)
    return out


# revision 34
# speedup vs baseline: 1.0056x; 1.0054x over previous
"""Trainium2 Bass kernel for conv-qkv rank-1 attention (bf16 pipeline).

out = gamma * q * sum(k*v) + x, where q,k,v are per-time-slice 3x3 convs
(C=64 -> C=64) of x [B=8, C=64, T=16, W=64, H=64].

Sharding: data-parallel over B across 8 cores (1 example/core), conv
weights replicated. No cross-core communication.

Per-core schedule: T slices in pairs; slice t on SBUF partitions 0-63,
slice t+1 on 64-127 -> two concurrent PE row-group chains (K=64), which
maxes the array fill rate (1 col/cycle/chain). All matmuls are uniform
64x128 stationaries (geometry changes stall the array ~300ns).
Everything streams bf16, PSUM accumulates f32:
  - x is staged twice (interior at even and odd column offsets) so every
    3x3 tap window is 4B-aligned -- unaligned bf16 moving operands cost
    ~20% fill rate.
  - chain-lo stationary [Wq|Wk] (q_t -> psum parts 0-63, aligned with
    x_t), chain-hi [Wk|Wq]; v uses zero-padded [0|Wv] / [Wv|0] into two
    psum tiles (4 tiles = 8 banks, double buffered).
  - Biases fold into the PSUM->SBUF evictions (Identity activation with
    per-partition bias), which also downcast to bf16. No bias tap.
  - kv mult+pixel-sum: one DVE STT per block half with hw accumulator.
  - out = q*(gamma*s) + x: 2-block-chunk DVE STTs, bf16 in/out; host
    upcasts. The writeback of pair p overlaps pair p+1's matmuls on the
    DVE queue. Host-padded x keeps loads as single contiguous DMAs.
"""

import numpy as np
import ml_dtypes

import concourse.bacc as bacc
import concourse.bass as bass
import concourse.mybir as mybir
import concourse.tile as tile
from concourse import bass_utils

F32 = mybir.dt.float32
BF16 = mybir.dt.bfloat16
ALU = mybir.AluOpType
ACT = mybir.ActivationFunctionType
NPBF16 = np.dtype(ml_dtypes.bfloat16)

B, C, T, W, H = 8, 64, 16, 64, 64
WP, HP = W + 2, H + 4            # pad rows [1,65); cols [2,66) / [3,67)
NPAIR = T // 2
RB = 8                           # W-rows per pixel block
NBLK = W // RB
BN = RB * H                      # moving free dim per matmul (512)
NTAP = 9
QC = 8                           # blocks per out-writeback chunk


def _pack_weights(wq, wk, wv):
    def taps(w):  # [O, I, 1, 3, 3] -> [I, 9, O]
        return np.ascontiguousarray(
            np.asarray(w, np.float32).reshape(C, C, 9).transpose(1, 2, 0))

    wq_t, wk_t, wv_t = taps(wq), taps(wk), taps(wv)
    wqk = np.zeros((128, NTAP, 128), np.float32)
    wqk[0:64, :, 0:64] = wq_t
    wqk[0:64, :, 64:128] = wk_t
    wqk[64:128, :, 0:64] = wk_t
    wqk[64:128, :, 64:128] = wq_t
    # v: chain-lo -> psum parts 64-127 (with k_t), chain-hi -> parts 0-63
    wv2 = np.zeros((128, NTAP, 128), np.float32)
    wv2[0:64, :, 64:128] = wv_t
    wv2[64:128, :, 0:64] = wv_t
    return wqk.astype(NPBF16), wv2.astype(NPBF16)


def _emit(nc, tc, xe_d, xo_d, wqk_d, wv_d, gam_d, blo_d, bhi_d, bvv_d,
          out_d, ctx):
    const = ctx.enter_context(tc.tile_pool(name="const", bufs=1))
    state = ctx.enter_context(tc.tile_pool(name="state", bufs=1))
    psum = ctx.enter_context(
        tc.tile_pool(name="psum", bufs=2, space=bass.MemorySpace.PSUM))
    vpool = ctx.enter_context(tc.tile_pool(name="vpool", bufs=3))

    wqk_t = const.tile([128, NTAP, 128], BF16, tag="wqk")
    wv_t = const.tile([128, NTAP, 128], BF16, tag="wv")
    gam_t = const.tile([128, 1], F32, tag="gam")
    blo_t = const.tile([128, 1], F32, tag="blo")
    bhi_t = const.tile([128, 1], F32, tag="bhi")
    bvv_t = const.tile([128, 1], F32, tag="bvv")

    nc.sync.dma_start(wqk_t[:], wqk_d[:])

    xe = [state.tile([128, WP, HP], BF16, tag=f"xe{i}", name=f"xe{i}")
          for i in range(3)]
    xo = [state.tile([128, WP, HP], BF16, tag=f"xo{i}", name=f"xo{i}")
          for i in range(3)]
    qk_lo = [state.tile([128, NBLK, BN], BF16, tag=f"qlo{i}", name=f"qlo{i}")
             for i in range(2)]
    qk_hi = [state.tile([128, NBLK, BN], BF16, tag=f"qhi{i}", name=f"qhi{i}")
             for i in range(2)]
    ot = [state.tile([128, NBLK, BN], BF16, tag=f"ot{i}", name=f"ot{i}")
          for i in range(2)]
    scr = state.tile([128, BN], BF16, tag="scr")
    sacc = [state.tile([128, NBLK], F32, tag=f"sa{i}", name=f"sa{i}")
            for i in range(2)]
    sful = [state.tile([128, 1], F32, tag=f"sf{i}", name=f"sf{i}")
            for i in range(2)]
    gsw = [state.tile([128, 1], F32, tag=f"gw{i}", name=f"gw{i}")
           for i in range(2)]

    def load_pair(p):
        te, to = xe[p % 3], xo[p % 3]
        nc.sync.dma_start(te[0:64], xe_d[2 * p])
        nc.sync.dma_start(te[64:128], xe_d[2 * p + 1])
        nc.sync.dma_start(to[0:64], xo_d[2 * p])
        nc.sync.dma_start(to[64:128], xo_d[2 * p + 1])

    # pair 0 loads in row-chunks in need-order: block j consumes padded
    # rows j*8..j*8+9, so the stream can start after ~0.6MB instead of
    # waiting for all 2.9MB of startup transfers (~200GB/s effective)
    p0_bounds = [(0, 18), (18, 34), (34, 50), (50, 66)]
    for ci, (r0, r1) in enumerate(p0_bounds):
        nc.sync.dma_start(xo[0][0:64, r0:r1, :], xo_d[0, :, r0:r1, :])
        nc.sync.dma_start(xo[0][64:128, r0:r1, :], xo_d[1, :, r0:r1, :])
        nc.sync.dma_start(xe[0][0:64, r0:r1, :], xe_d[0, :, r0:r1, :])
        nc.sync.dma_start(xe[0][64:128, r0:r1, :], xe_d[1, :, r0:r1, :])
        if ci == 0:
            nc.sync.dma_start(wv_t[:], wv_d[:])    # needed at slot 9
    nc.sync.dma_start(gam_t[:], gam_d[:])
    nc.sync.dma_start(blo_t[:], blo_d[:])
    nc.sync.dma_start(bhi_t[:], bhi_d[:])
    nc.sync.dma_start(bvv_t[:], bvv_d[:])
    if NPAIR > 1:
        load_pair(1)

    for p in range(NPAIR):
        pb = p % 2
        xe_, xo_ = xe[p % 3], xo[p % 3]
        qlo_, qhi_, ot_ = qk_lo[pb], qk_hi[pb], ot[pb]

        if p + 2 < NPAIR:
            load_pair(p + 2)

        for j in range(NBLK):
            pqk_lo = psum.tile([128, BN], F32, tag="pqk_lo")
            pqk_hi = psum.tile([128, BN], F32, tag="pqk_hi")
            pvv_lo = psum.tile([128, BN], F32, tag="pvv_lo", name="pvv_lo")
            pvv_hi = psum.tile([128, BN], F32, tag="pvv_hi", name="pvv_hi")

            def rhs(half, tap):
                dy, dx = tap // 3, tap % 3
                r0 = j * RB + dy
                base = 64 * half
                if dx == 1:
                    return xe_[base:base + 64, r0:r0 + RB, 2:2 + H]
                if dx == 0:
                    return xo_[base:base + 64, r0:r0 + RB, 2:2 + H]
                return xo_[base:base + 64, r0:r0 + RB, 4:4 + H]

            for tap in range(NTAP):
                st, sp = tap == 0, tap == NTAP - 1
                nc.tensor.matmul(pqk_lo[:, :], wqk_t[0:64, tap, :],
                                 rhs(0, tap), start=st, stop=sp)
                nc.tensor.matmul(pqk_hi[:, :], wqk_t[64:128, tap, :],
                                 rhs(1, tap), start=st, stop=sp)
            for tap in range(NTAP):
                st, sp = tap == 0, tap == NTAP - 1
                nc.tensor.matmul(pvv_lo[:, :], wv_t[0:64, tap, :],
                                 rhs(0, tap), start=st, stop=sp)
                nc.tensor.matmul(pvv_hi[:, :], wv_t[64:128, tap, :],
                                 rhs(1, tap), start=st, stop=sp)

            # evict psum -> bf16 sbuf, adding conv biases (per-partition)
            nc.scalar.activation(qlo_[:, j, :], pqk_lo[:, :], ACT.Identity,
                                 bias=blo_t[:, 0:1])
            nc.scalar.activation(qhi_[:, j, :], pqk_hi[:, :], ACT.Identity,
                                 bias=bhi_t[:, 0:1])
            vsb = vpool.tile([128, BN], BF16, tag="vsb", name="vsb")
            nc.scalar.activation(vsb[64:128, :], pvv_lo[64:128, :],
                                 ACT.Identity, bias=bvv_t[64:128, 0:1])
            nc.scalar.activation(vsb[0:64, :], pvv_hi[0:64, :],
                                 ACT.Identity, bias=bvv_t[0:64, 0:1])

            # fused k*v multiply + pixel-sum
            nc.vector.scalar_tensor_tensor(
                out=scr[64:128, :], in0=qlo_[64:128, j, :], scalar=1.0,
                in1=vsb[64:128, :], op0=ALU.mult, op1=ALU.mult,
                accum_out=sacc[pb][64:128, j:j + 1])
            nc.vector.scalar_tensor_tensor(
                out=scr[0:64, :], in0=qhi_[0:64, j, :], scalar=1.0,
                in1=vsb[0:64, :], op0=ALU.mult, op1=ALU.mult,
                accum_out=sacc[pb][0:64, j:j + 1])

        nc.vector.reduce_sum(sful[pb][:, :], sacc[pb][:, :],
                             axis=mybir.AxisListType.X)
        nc.vector.tensor_scalar_mul(sful[pb][:, :], sful[pb][:, :],
                                    gam_t[:, 0:1])
        # gs accumulated on k's partitions = complement of q's: swap halves
        nc.scalar.copy(gsw[pb][0:64, :], sful[pb][64:128, :])
        nc.scalar.copy(gsw[pb][64:128, :], sful[pb][0:64, :])

        for m in range(0, NBLK, QC):
            # out = q * (gamma*s) + x, fused (bf16), QC blocks per op
            r0 = 1 + m * RB
            nc.vector.scalar_tensor_tensor(
                out=ot_[0:64, m:m + QC, :],
                in0=qlo_[0:64, m:m + QC, :],
                scalar=gsw[pb][0:64, 0:1],
                in1=xe_[0:64, r0:r0 + QC * RB, 2:2 + H],
                op0=ALU.mult, op1=ALU.add)
            nc.vector.scalar_tensor_tensor(
                out=ot_[64:128, m:m + QC, :],
                in0=qhi_[64:128, m:m + QC, :],
                scalar=gsw[pb][64:128, 0:1],
                in1=xe_[64:128, r0:r0 + QC * RB, 2:2 + H],
                op0=ALU.mult, op1=ALU.add)
            nc.sync.dma_start(out_d[2 * p, :, m * BN:(m + QC) * BN],
                              ot_[0:64, m:m + QC, :])
            nc.sync.dma_start(
                out_d[2 * p + 1, :, m * BN:(m + QC) * BN],
                ot_[64:128, m:m + QC, :])


_CACHE = {}


def _build():
    if "nc" in _CACHE:
        return _CACHE["nc"]
    nc = bacc.Bacc("TRN2", target_bir_lowering=False, debug=False,
                   enable_asserts=False, num_devices=8)
    xe_d = nc.dram_tensor("xe16", (T, C, WP, HP), BF16,
                          kind="ExternalInput").ap()
    xo_d = nc.dram_tensor("xo16", (T, C, WP, HP), BF16,
                          kind="ExternalInput").ap()
    wqk_d = nc.dram_tensor("wqk", (128, NTAP, 128), BF16,
                           kind="ExternalInput").ap()
    wv_d = nc.dram_tensor("wv2", (128, NTAP, 128), BF16,
                          kind="ExternalInput").ap()
    gam_d = nc.dram_tensor("gamma_bc", (128, 1), F32,
                           kind="ExternalInput").ap()
    blo_d = nc.dram_tensor("b_lo", (128, 1), F32, kind="ExternalInput").ap()
    bhi_d = nc.dram_tensor("b_hi", (128, 1), F32, kind="ExternalInput").ap()
    bvv_d = nc.dram_tensor("b_vv", (128, 1), F32, kind="ExternalInput").ap()
    out_d = nc.dram_tensor("out", (T, C, W * H), BF16,
                           kind="ExternalOutput").ap()
    from contextlib import ExitStack
    with tile.TileContext(nc) as tc, ExitStack() as ctx:
        _emit(nc, tc, xe_d, xo_d, wqk_d, wv_d, gam_d, blo_d, bhi_d, bvv_d,
              out_d, ctx)
    nc.compile()
    _CACHE["nc"] = nc
    return nc


def run_spmd(x, wq, wk, wv, bq, bk, bv, gamma, trace=False, **kw):
    nc = _build()
    wqk, wv2 = _pack_weights(wq, wk, wv)
    bq = np.asarray(bq, np.float32).reshape(C)
    bk = np.asarray(bk, np.float32).reshape(C)
    bv = np.asarray(bv, np.float32).reshape(C)
    blo = np.concatenate([bq, bk]).reshape(128, 1)
    bhi = np.concatenate([bk, bq]).reshape(128, 1)
    bvv = np.concatenate([bv, bv]).reshape(128, 1)
    gam = np.full((128, 1), np.float32(np.asarray(gamma).reshape(-1)[0]),
                  np.float32)
    x = np.asarray(x, np.float32)
    in_maps = []
    for b in range(B):
        xt = x[b].transpose(1, 0, 2, 3).astype(NPBF16)
        xe = np.zeros((T, C, WP, HP), NPBF16)
        xe[:, :, 1:1 + W, 2:2 + H] = xt
        xo = np.zeros((T, C, WP, HP), NPBF16)
        xo[:, :, 1:1 + W, 3:3 + H] = xt
        in_maps.append({"xe16": xe, "xo16": xo, "wqk": wqk, "wv2": wv2,
                        "gamma_bc": gam, "b_lo": blo, "b_hi": bhi,
                        "b_vv": bvv})
    res = bass_utils.run_bass_kernel_spmd(
        nc, in_maps, core_ids=list(range(B)), trace=trace, **kw)
    out = np.stack(
        [res.results[b]["out"].astype(np.float32)
         .reshape(T, C, W, H).transpose(1, 0, 2, 3) for b in range(B)],
        axis=0)
    return out, res


def kernel(x, wq, wk, wv, bq, bk, bv, gamma):
    out, _ = run_spmd(x, wq, wk, wv, bq, bk, bv, gamma)
    return out
